# revision 24
# baseline (speedup 1.0000x reference)
"""Contrastive loss (NT-Xent style) Trainium2 kernel, symmetric-half version.

Reference math: z = concat(x1, x2); zn = z / max(||z||, eps);
sim = zn @ zn.T / beta; loss = mean_i(log(sum_{j!=i} exp(sim_ij)) - pos_i)
with pos_i = sim[i, (i + N) mod 2N].

Sharding: rows of the 2N x 2N similarity matrix split across 8 cores (1024
rows each). Inputs are rotated by -1024*c rows per core so the SPMD program
is identical everywhere: local rows are always [0, 1024), the sim diagonal
falls in local column group 0, and the positive-pair column in group 4.

Symmetry: exp(sim) is symmetric, so each core computes only column groups
0..4 of its row band (with a 128-col staircase on groups 0 and 4: subtiles
u >= t). Row sums of the computed half come free from the ACT accum; the
mirrored half is recovered from COLUMN sums of exp(sim), accumulated in
bf16 on the vector engine. The host scatter-adds the per-core column-sum
pieces into the global denominator, takes log, and means.

Device data flow per core:
  DMA in (spread over 4 engine queues): fp8-e4m3 normalized transposed
  embeddings laid out [128, group, k, 1024] for DoubleRow matmuls.
  per M-tile t (8x): fp8 DoubleRow matmuls (full K=256 per instruction)
  into 2048-col PSUM superchunks; the self-similarity diagonal is masked
  by accumulating -240*240*I via an extra small matmul; exp on ScalarE
  (scale=1/beta, accum_out = row sums) writes bf16 E to SBUF; the vector
  engine adds E into the column accumulator (strict-upper subtiles only
  on groups 0/4 so no element is counted twice).

Host does layout-only prep (concat/roll/slice/transpose/cast), the
normalization of z, and the final den assembly + log + mean.
"""

import numpy as np
from contextlib import ExitStack

import ml_dtypes

import concourse.bass as bass
import concourse.tile as tile
from concourse import bacc, mybir
from concourse.bass_utils import run_bass_kernel_spmd

BETA = 0.08
EPS = 1e-8
TWO_N = 8192
D = 256
N_CORES = 8
RPC = TWO_N // N_CORES          # 1024 rows per core
MT = RPC // 128                 # 8 M-tiles per core
NGRP = 5                        # column groups 0..4
NCOLS = NGRP * RPC              # 5120 local columns
GA_END = 4 * RPC                # end of run A (groups 0..3)
SC = 2048                       # superchunk width (4 PSUM banks)
CHUNK = 512                     # matmul output free dim
MASK = -240.0 * 240.0           # diagonal mask value from the fp8 mask matmul

F32 = mybir.dt.float32
BF16 = mybir.dt.bfloat16
FP8 = mybir.dt.float8e4
AF = mybir.ActivationFunctionType
ALU = mybir.AluOpType
AX = mybir.AxisListType
DR = mybir.MatmulPerfMode.DoubleRow

TRACE = False
LAST_EXEC_NS = None
LAST_RESULTS = None

_cached_nc = None


def _build():
    nc = bacc.Bacc(
        "TRN2", target_bir_lowering=False, debug=False, num_devices=N_CORES
    )
    # znt8[p, g, k, j] = zn[(1024*core + 1024*g + j) % 2N, 128*k + p] as fp8
    znt8 = nc.dram_tensor(
        "znt8", [128, NGRP, 2, RPC], FP8, kind="ExternalInput"
    ).ap()
    eye = nc.dram_tensor("eye", [128, 128], F32, kind="ExternalInput").ap()
    eye8n = nc.dram_tensor("eye8n", [128, 128], FP8, kind="ExternalInput").ap()
    # eyesh[q, 384 + q'] = 240 iff q' == q: sliced at [384-pad, 896-pad) it
    # puts 240 at out-col pad+q, so the mask matmul writes a full aligned
    # 512-wide PSUM slice with -240*240 only on the diagonal subtile.
    eyesh = nc.dram_tensor("eyesh", [128, 896], FP8, kind="ExternalInput").ap()
    # ramp[q, c] = 240 iff c < 384: sliced at [384-pad, 896-pad) it makes
    # the first `pad` output columns -240*240 for EVERY row — used to kill
    # the alignment-junk columns in the middle of a packed superchunk so
    # their exp contributes 0 to the row sums.
    ramp = nc.dram_tensor("ramp", [128, 896], FP8, kind="ExternalInput").ap()
    den_out = nc.dram_tensor("den_direct", [128, MT], F32, kind="ExternalOutput").ap()
    pos_out = nc.dram_tensor("pos", [128, MT], F32, kind="ExternalOutput").ap()
    cs_out = nc.dram_tensor("colsum", [128, NCOLS], BF16, kind="ExternalOutput").ap()

    with tile.TileContext(nc) as tc, ExitStack() as ctx:
        const_pool = ctx.enter_context(tc.tile_pool(name="const", bufs=1))
        zpool = ctx.enter_context(tc.tile_pool(name="zp", bufs=1))
        acc_pool = ctx.enter_context(tc.tile_pool(name="acc", bufs=1))
        est_pool = ctx.enter_context(tc.tile_pool(name="est", bufs=3))
        small = ctx.enter_context(tc.tile_pool(name="small", bufs=1))
        scr = ctx.enter_context(tc.tile_pool(name="scr", bufs=2))
        mm_psum = ctx.enter_context(tc.tile_pool(name="mm", bufs=2, space="PSUM"))

        # znT in DoubleRow layout, one tile per column group: [128, 2, 1024].
        # One DMA per group (2 KB/partition), spread over sync+gpsimd.
        # Group 0 goes absolutely first (the first matmuls need it), then
        # the mask constants, then the rest in consumption order.
        znt = [
            zpool.tile([128, 2, RPC], FP8, tag=f"znt{g}", name=f"znt{g}")
            for g in range(NGRP)
        ]
        nc.sync.dma_start(znt[0][:, :, :], znt8[:, 0, :, :])
        eyesh_sb = const_pool.tile([128, 896], FP8, tag="eyesh")
        nc.gpsimd.dma_start(eyesh_sb[:], eyesh[:, :])
        eye8n_sb = const_pool.tile([128, 128], FP8, tag="eye8n")
        nc.gpsimd.dma_start(eye8n_sb[:], eye8n[:, :])
        for g in range(1, NGRP):
            (nc.sync, nc.gpsimd)[g % 2].dma_start(
                znt[g][:, :, :], znt8[:, g, :, :]
            )
        ramp_sb = const_pool.tile([128, 896], FP8, tag="ramp")
        nc.sync.dma_start(ramp_sb[:], ramp[:, :])
        eye_sb = const_pool.tile([128, 128], F32, tag="eye")
        nc.gpsimd.dma_start(eye_sb[:], eye[:, :])

        # column-sum accumulators (split by last-writing M-tile so the
        # early-finalized pieces DMA out under the tail of the main loop)
        acc0 = acc_pool.tile([128, RPC], BF16, tag="acc0")       # g0 cols
        acc123 = acc_pool.tile([128, 3 * RPC], BF16, tag="acc123")
        acc4 = acc_pool.tile([128, RPC], BF16, tag="acc4")       # g4 cols
        nc.vector.memset(acc0[:], 0.0)
        nc.vector.memset(acc123[:], 0.0)
        nc.vector.memset(acc4[:], 0.0)

        den_all = small.tile([128, MT], F32, tag="den")
        pos = small.tile([128, MT], F32, tag="pos")

        for t in range(MT):
            # Every matmul writes one full 512-aligned PSUM slice (unaligned
            # or ragged PSUM writes corrupt the accumulation). Runs A
            # (g0 tail + g1..3) and B (g4 tail) are packed into one flat
            # 512-chunk list: A's alignment junk leads and is skipped by
            # ACT; B's alignment junk sits mid-stream and is masked to
            # -inf by the ramp matmul so its exp contributes 0.
            run_loA = 128 * t
            padA = run_loA % CHUNK
            a_loA = run_loA - padA
            run_loB = GA_END + 128 * t
            padB = run_loB % CHUNK
            a_loB = run_loB - padB
            lenA_al = GA_END - a_loA
            lenA = GA_END - run_loA
            lenB = NCOLS - run_loB
            chunks = list(range(a_loA, GA_END, CHUNK)) + list(
                range(a_loB, NCOLS, CHUNK)
            )
            total = CHUNK * len(chunks)

            denp = scr.tile([128, 4], F32, tag=f"denp{t}")
            est = est_pool.tile([128, NCOLS], BF16, tag="est")
            n_sc = 0
            for sc_i0 in range(0, len(chunks), SC // CHUNK):
                sc_chunks = chunks[sc_i0 : sc_i0 + SC // CHUNK]
                sc_off = CHUNK * sc_i0
                sc_len = CHUNK * len(sc_chunks)
                pg = mm_psum.tile([128, SC], F32, tag="mm")
                for j, col in enumerate(sc_chunks):
                    g = col // RPC
                    mask = None
                    if col == a_loA:
                        mask = eyesh_sb[:, 384 - padA : 896 - padA]
                    elif col == a_loB and padB > 0:
                        mask = ramp_sb[:, 384 - padB : 896 - padB]
                    nc.tensor.matmul(
                        pg[:, CHUNK * j : CHUNK * (j + 1)],
                        znt[0][:, :, 128 * t : 128 * (t + 1)],
                        znt[g][:, :, col - RPC * g : col - RPC * g + CHUNK],
                        start=True,
                        stop=mask is None,
                        perf_mode=DR,
                    )
                    if mask is not None:
                        nc.tensor.matmul(
                            pg[:, CHUNK * j : CHUNK * (j + 1)],
                            eye8n_sb[:],
                            mask,
                            start=False,
                            stop=True,
                            skip_group_check=True,
                        )
                skip = padA if sc_i0 == 0 else 0
                nc.scalar.activation(
                    est[:, sc_off - padA + skip : sc_off - padA + sc_len],
                    pg[:, skip:sc_len],
                    AF.Exp,
                    scale=1.0 / BETA,
                    accum_out=denp[:, n_sc : n_sc + 1],
                )
                if sc_off <= lenA_al + padB < sc_off + sc_len:
                    # positive pair on the diagonal of g4's first subtile
                    roffB = lenA_al + padB - sc_off
                    pdump = scr.tile([128, 128], F32, tag="posdump")
                    nc.vector.scalar_tensor_tensor(
                        out=pdump[:],
                        in0=pg[:, roffB : roffB + 128],
                        scalar=1.0 / BETA,
                        in1=eye_sb[:],
                        op0=ALU.mult,
                        op1=ALU.mult,
                        accum_out=pos[:, t : t + 1],
                    )
                # mirror-half column sums for the est range this SC just
                # produced (issued per-SC so the adds pipeline with the
                # remaining exps instead of bunching at the end of t).
                # est layout: [0, lenA) = cols [run_loA, GA_END);
                # [lenA, lenA+padB) = zeros; [lenA+padB, ...) = g4 cols.
                e_lo = sc_off - padA if sc_off else 0
                e_hi = sc_off - padA + sc_len
                for alo, ahi, acc_t, aoff in (
                    (128, RPC - run_loA, acc0, run_loA),
                    (RPC - run_loA, lenA, acc123, run_loA - RPC),
                    (lenA + padB + 128, lenA + padB + lenB, acc4,
                     run_loB - lenA - padB - GA_END),
                ):
                    lo, hi = max(alo, e_lo), min(ahi, e_hi)
                    if lo < hi:
                        nc.vector.tensor_add(
                            acc_t[:, lo + aoff : hi + aoff],
                            acc_t[:, lo + aoff : hi + aoff],
                            est[:, lo:hi],
                        )
                n_sc += 1
            nc.vector.tensor_reduce(
                den_all[:, t : t + 1], denp[:, 0:n_sc], axis=AX.X, op=ALU.add
            )

        # column accumulators go out raw; host does the partition-sum.
        # acc0/acc4 finalize one M-tile early; the g1-3 piece is split
        # across three queues so the tail transfers run in parallel.
        nc.gpsimd.dma_start(cs_out[:, 0:RPC], acc0[:])
        nc.gpsimd.dma_start(cs_out[:, GA_END:NCOLS], acc4[:])
        nc.sync.dma_start(cs_out[:, RPC : 2 * RPC], acc123[:, 0:RPC])
        nc.gpsimd.dma_start(cs_out[:, 2 * RPC : 3 * RPC], acc123[:, RPC : 2 * RPC])
        nc.scalar.dma_start(cs_out[:, 3 * RPC : GA_END], acc123[:, 2 * RPC :])
        nc.sync.dma_start(den_out[:, :], den_all[:])
        nc.sync.dma_start(pos_out[:, :], pos[:])

    nc.compile()
    return nc


def _get_nc():
    global _cached_nc
    if _cached_nc is None:
        _cached_nc = _build()
    return _cached_nc


def kernel(x1: np.ndarray, x2: np.ndarray) -> np.ndarray:
    global LAST_EXEC_NS, LAST_RESULTS
    z = np.concatenate(
        [np.asarray(x1, dtype=np.float32), np.asarray(x2, dtype=np.float32)], axis=0
    )
    norms = np.sqrt(np.sum(z * z, axis=1, keepdims=True))
    zn = z / np.maximum(norms, EPS)

    fp8 = mybir.dt.np(FP8)
    eye = np.eye(128, dtype=np.float32)
    eye8n = (-240.0 * eye).astype(fp8)
    eyesh = np.zeros((128, 896), dtype=np.float32)
    eyesh[np.arange(128), 384 + np.arange(128)] = 240.0
    eyesh = eyesh.astype(fp8)
    ramp = np.zeros((128, 896), dtype=np.float32)
    ramp[:, :384] = 240.0
    ramp = ramp.astype(fp8)
    in_maps = []
    for c in range(N_CORES):
        zc = np.roll(zn, -RPC * c, axis=0)[:NCOLS]
        # [d, col] -> [p, group, k, j] with d = 128*k + p, col = 1024*g + j
        znt = zc.T.astype(fp8).reshape(2, 128, NGRP, RPC)
        znt8 = np.ascontiguousarray(znt.transpose(1, 2, 0, 3))
        in_maps.append(
            {"znt8": znt8, "eye": eye, "eye8n": eye8n, "eyesh": eyesh, "ramp": ramp}
        )
    nc = _get_nc()
    res = run_bass_kernel_spmd(nc, in_maps, list(range(N_CORES)), trace=TRACE)
    LAST_EXEC_NS = res.exec_time_ns
    LAST_RESULTS = res

    # ---- gather / unshard: assemble global denominator & positives ----
    den = np.zeros(TWO_N, dtype=np.float64)
    pos = np.zeros(TWO_N, dtype=np.float64)
    idx = np.arange(NCOLS)
    for c in range(N_CORES):
        r = res.results[c]
        rows = RPC * c + np.arange(RPC)
        den[rows] += r["den_direct"].astype(np.float64).T.reshape(-1)
        pos[rows] += r["pos"].astype(np.float64).T.reshape(-1)
        den[(RPC * c + idx) % TWO_N] += r["colsum"].astype(np.float64).sum(axis=0)
    loss = np.mean(np.log(den) - pos)
    return np.array(loss, dtype=np.float32)


# revision 28
# speedup vs baseline: 1.0754x; 1.0754x over previous
"""Contrastive loss (NT-Xent style) Trainium2 kernel, symmetric-half version.

Reference math: z = concat(x1, x2); zn = z / max(||z||, eps);
sim = zn @ zn.T / beta; loss = mean_i(log(sum_{j!=i} exp(sim_ij)) - pos_i)
with pos_i = sim[i, (i + N) mod 2N].

Sharding: rows of the 2N x 2N similarity matrix split across 8 cores (1024
rows each). Inputs are rotated by -1024*c rows per core so the SPMD program
is identical everywhere: local rows are always [0, 1024), the sim diagonal
falls in local column group 0, and the positive-pair column in group 4.

Symmetry: exp(sim) is symmetric, so each core computes only column groups
0..4 of its row band (with a 128-col staircase on groups 0 and 4: subtiles
u >= t). Row sums of the computed half come free from the ACT accum; the
mirrored half is recovered from COLUMN sums of exp(sim), accumulated in
bf16 on the vector engine. The host scatter-adds the per-core column-sum
pieces into the global denominator, takes log, and means.

Device data flow per core:
  DMA in (spread over 4 engine queues): fp8-e4m3 normalized transposed
  embeddings laid out [128, group, k, 1024] for DoubleRow matmuls.
  per M-tile t (8x): fp8 DoubleRow matmuls (full K=256 per instruction)
  into 2048-col PSUM superchunks; the self-similarity diagonal is masked
  by accumulating -240*240*I via an extra small matmul; exp on ScalarE
  (scale=1/beta, accum_out = row sums) writes bf16 E to SBUF; the vector
  engine adds E into the column accumulator (strict-upper subtiles only
  on groups 0/4 so no element is counted twice).

Host does layout-only prep (concat/roll/slice/transpose/cast), the
normalization of z, and the final den assembly + log + mean.
"""

import numpy as np
from contextlib import ExitStack

import ml_dtypes

import concourse.bass as bass
import concourse.tile as tile
from concourse import bacc, mybir
from concourse.bass_utils import run_bass_kernel_spmd

BETA = 0.08
EPS = 1e-8
TWO_N = 8192
D = 256
N_CORES = 8
RPC = TWO_N // N_CORES          # 1024 rows per core
MT = RPC // 128                 # 8 M-tiles per core
NGRP = 5                        # column groups 0..4
NCOLS = NGRP * RPC              # 5120 local columns
GA_END = 4 * RPC                # end of run A (groups 0..3)
SC = 2048                       # superchunk width (4 PSUM banks)
CHUNK = 512                     # matmul output free dim
MASK = -240.0 * 240.0           # diagonal mask value from the fp8 mask matmul

F32 = mybir.dt.float32
BF16 = mybir.dt.bfloat16
FP8 = mybir.dt.float8e4
AF = mybir.ActivationFunctionType
ALU = mybir.AluOpType
AX = mybir.AxisListType
DR = mybir.MatmulPerfMode.DoubleRow

TRACE = False
LAST_EXEC_NS = None
LAST_RESULTS = None

_cached_nc = None


def _build():
    nc = bacc.Bacc(
        "TRN2", target_bir_lowering=False, debug=False, num_devices=N_CORES
    )
    # znt8[p, g, k, j] = zn[(1024*core + 1024*g + j) % 2N, 128*k + p] as fp8
    znt8 = nc.dram_tensor(
        "znt8", [128, NGRP, 2, RPC], FP8, kind="ExternalInput"
    ).ap()
    eye = nc.dram_tensor("eye", [128, 128], F32, kind="ExternalInput").ap()
    eye8n = nc.dram_tensor("eye8n", [128, 128], FP8, kind="ExternalInput").ap()
    # eyesh[q, 384 + q'] = 240 iff q' == q: sliced at [384-pad, 896-pad) it
    # puts 240 at out-col pad+q, so the mask matmul writes a full aligned
    # 512-wide PSUM slice with -240*240 only on the diagonal subtile.
    eyesh = nc.dram_tensor("eyesh", [128, 896], FP8, kind="ExternalInput").ap()
    # ramp[q, c] = 240 iff c < 384: sliced at [384-pad, 896-pad) it makes
    # the first `pad` output columns -240*240 for EVERY row — used to kill
    # the alignment-junk columns in the middle of a packed superchunk so
    # their exp contributes 0 to the row sums.
    ramp = nc.dram_tensor("ramp", [128, 896], FP8, kind="ExternalInput").ap()
    den_out = nc.dram_tensor("den_direct", [128, MT], F32, kind="ExternalOutput").ap()
    pos_out = nc.dram_tensor("pos", [128, MT], F32, kind="ExternalOutput").ap()
    cs_out = nc.dram_tensor("colsum", [128, NCOLS], BF16, kind="ExternalOutput").ap()

    with tile.TileContext(nc) as tc, ExitStack() as ctx:
        const_pool = ctx.enter_context(tc.tile_pool(name="const", bufs=1))
        zpool = ctx.enter_context(tc.tile_pool(name="zp", bufs=1))
        acc_pool = ctx.enter_context(tc.tile_pool(name="acc", bufs=1))
        est_pool = ctx.enter_context(tc.tile_pool(name="est", bufs=3))
        small = ctx.enter_context(tc.tile_pool(name="small", bufs=1))
        scr = ctx.enter_context(tc.tile_pool(name="scr", bufs=2))
        mm_psum = ctx.enter_context(tc.tile_pool(name="mm", bufs=2, space="PSUM"))

        # znT in DoubleRow layout, one tile per column group: [128, 2, 1024].
        # One DMA per group (2 KB/partition), spread over sync+gpsimd.
        # Group 0 goes absolutely first (the first matmuls need it), then
        # the mask constants, then the rest in consumption order.
        znt = [
            zpool.tile([128, 2, RPC], FP8, tag=f"znt{g}", name=f"znt{g}")
            for g in range(NGRP)
        ]
        nc.sync.dma_start(znt[0][:, :, :], znt8[:, 0, :, :])
        eyesh_sb = const_pool.tile([128, 896], FP8, tag="eyesh")
        nc.gpsimd.dma_start(eyesh_sb[:], eyesh[:, :])
        eye8n_sb = const_pool.tile([128, 128], FP8, tag="eye8n")
        nc.gpsimd.dma_start(eye8n_sb[:], eye8n[:, :])
        for g in range(1, NGRP):
            (nc.gpsimd, nc.sync)[g % 2].dma_start(
                znt[g][:, :, :], znt8[:, g, :, :]
            )
        ramp_sb = const_pool.tile([128, 896], FP8, tag="ramp")
        nc.sync.dma_start(ramp_sb[:], ramp[:, :])
        eye_sb = const_pool.tile([128, 128], F32, tag="eye")
        nc.gpsimd.dma_start(eye_sb[:], eye[:, :])

        # column-sum accumulators (split by last-writing M-tile so the
        # early-finalized pieces DMA out under the tail of the main loop).
        # No memset: M-tile 0 covers every region, so its pieces are
        # written with tensor_copy; the never-touched 128-col edges are
        # ignored by the host.
        acc0 = acc_pool.tile([128, RPC], BF16, tag="acc0")       # g0 cols
        acc123 = acc_pool.tile([128, 3 * RPC], BF16, tag="acc123")
        acc4 = acc_pool.tile([128, RPC], BF16, tag="acc4")       # g4 cols

        den_all = small.tile([128, MT], F32, tag="den")
        pos = small.tile([128, MT], F32, tag="pos")

        for t in range(MT):
            # Every matmul writes one full 512-aligned PSUM slice (unaligned
            # or ragged PSUM writes corrupt the accumulation). Runs A
            # (g0 tail + g1..3) and B (g4 tail) are packed into one flat
            # 512-chunk list: A's alignment junk leads and is skipped by
            # ACT; B's alignment junk sits mid-stream and is masked to
            # -inf by the ramp matmul so its exp contributes 0.
            run_loA = 128 * t
            padA = run_loA % CHUNK
            a_loA = run_loA - padA
            run_loB = GA_END + 128 * t
            padB = run_loB % CHUNK
            a_loB = run_loB - padB
            lenA_al = GA_END - a_loA
            lenA = GA_END - run_loA
            lenB = NCOLS - run_loB
            chunks = list(range(a_loA, GA_END, CHUNK)) + list(
                range(a_loB, NCOLS, CHUNK)
            )
            total = CHUNK * len(chunks)

            denp = scr.tile([128, 4], F32, tag=f"denp{t}")
            est = est_pool.tile([128, NCOLS], BF16, tag="est")
            n_sc = 0
            # smaller first superchunk on the very first M-tile so the
            # scalar engine starts as soon as group 0 lands
            sc_grid = [2, 4, 4] if t == 0 else [4] * 3
            sc_starts = [0]
            for w in sc_grid:
                sc_starts.append(sc_starts[-1] + w)
            for sc_i0, sc_i1 in zip(sc_starts, sc_starts[1:]):
                sc_chunks = chunks[sc_i0:sc_i1]
                if not sc_chunks:
                    break
                sc_off = CHUNK * sc_i0
                sc_len = CHUNK * len(sc_chunks)
                pg = mm_psum.tile([128, SC], F32, tag="mm")
                for j, col in enumerate(sc_chunks):
                    g = col // RPC
                    mask = None
                    if col == a_loA:
                        mask = eyesh_sb[:, 384 - padA : 896 - padA]
                    elif col == a_loB and padB > 0:
                        mask = ramp_sb[:, 384 - padB : 896 - padB]
                    nc.tensor.matmul(
                        pg[:, CHUNK * j : CHUNK * (j + 1)],
                        znt[0][:, :, 128 * t : 128 * (t + 1)],
                        znt[g][:, :, col - RPC * g : col - RPC * g + CHUNK],
                        start=True,
                        stop=mask is None,
                        perf_mode=DR,
                    )
                    if mask is not None:
                        nc.tensor.matmul(
                            pg[:, CHUNK * j : CHUNK * (j + 1)],
                            eye8n_sb[:],
                            mask,
                            start=False,
                            stop=True,
                            skip_group_check=True,
                        )
                skip = padA if sc_i0 == 0 else 0
                nc.scalar.activation(
                    est[:, sc_off - padA + skip : sc_off - padA + sc_len],
                    pg[:, skip:sc_len],
                    AF.Exp,
                    scale=1.0 / BETA,
                    accum_out=denp[:, n_sc : n_sc + 1],
                )
                if sc_off <= lenA_al + padB < sc_off + sc_len:
                    # positive pair on the diagonal of g4's first subtile
                    roffB = lenA_al + padB - sc_off
                    pdump = scr.tile([128, 128], F32, tag="posdump")
                    nc.vector.scalar_tensor_tensor(
                        out=pdump[:],
                        in0=pg[:, roffB : roffB + 128],
                        scalar=1.0 / BETA,
                        in1=eye_sb[:],
                        op0=ALU.mult,
                        op1=ALU.mult,
                        accum_out=pos[:, t : t + 1],
                    )
                # mirror-half column sums for the est range this SC just
                # produced (issued per-SC so the adds pipeline with the
                # remaining exps instead of bunching at the end of t).
                # est layout: [0, lenA) = cols [run_loA, GA_END);
                # [lenA, lenA+padB) = zeros; [lenA+padB, ...) = g4 cols.
                e_lo = sc_off - padA if sc_off else 0
                e_hi = sc_off - padA + sc_len
                for alo, ahi, acc_t, aoff in (
                    (128, RPC - run_loA, acc0, run_loA),
                    (RPC - run_loA, lenA, acc123, run_loA - RPC),
                    (lenA + padB + 128, lenA + padB + lenB, acc4,
                     run_loB - lenA - padB - GA_END),
                ):
                    lo, hi = max(alo, e_lo), min(ahi, e_hi)
                    if lo < hi:
                        if t == 0:
                            nc.vector.tensor_copy(
                                acc_t[:, lo + aoff : hi + aoff], est[:, lo:hi]
                            )
                        else:
                            nc.vector.tensor_add(
                                acc_t[:, lo + aoff : hi + aoff],
                                acc_t[:, lo + aoff : hi + aoff],
                                est[:, lo:hi],
                            )
                n_sc += 1
            nc.vector.tensor_reduce(
                den_all[:, t : t + 1], denp[:, 0:n_sc], axis=AX.X, op=ALU.add
            )

        # column accumulators go out raw; host does the partition-sum.
        # acc0/acc4 finalize one M-tile early; the g1-3 piece is split
        # across three queues so the tail transfers run in parallel.
        nc.gpsimd.dma_start(cs_out[:, 0:RPC], acc0[:])
        nc.gpsimd.dma_start(cs_out[:, GA_END:NCOLS], acc4[:])
        nc.sync.dma_start(cs_out[:, RPC : 2 * RPC], acc123[:, 0:RPC])
        nc.gpsimd.dma_start(cs_out[:, 2 * RPC : 3 * RPC], acc123[:, RPC : 2 * RPC])
        nc.scalar.dma_start(cs_out[:, 3 * RPC : GA_END], acc123[:, 2 * RPC :])
        nc.sync.dma_start(den_out[:, :], den_all[:])
        nc.sync.dma_start(pos_out[:, :], pos[:])

    nc.compile()
    return nc


def _get_nc():
    global _cached_nc
    if _cached_nc is None:
        _cached_nc = _build()
    return _cached_nc


def kernel(x1: np.ndarray, x2: np.ndarray) -> np.ndarray:
    global LAST_EXEC_NS, LAST_RESULTS
    z = np.concatenate(
        [np.asarray(x1, dtype=np.float32), np.asarray(x2, dtype=np.float32)], axis=0
    )
    norms = np.sqrt(np.sum(z * z, axis=1, keepdims=True))
    zn = z / np.maximum(norms, EPS)

    fp8 = mybir.dt.np(FP8)
    eye = np.eye(128, dtype=np.float32)
    eye8n = (-240.0 * eye).astype(fp8)
    eyesh = np.zeros((128, 896), dtype=np.float32)
    eyesh[np.arange(128), 384 + np.arange(128)] = 240.0
    eyesh = eyesh.astype(fp8)
    ramp = np.zeros((128, 896), dtype=np.float32)
    ramp[:, :384] = 240.0
    ramp = ramp.astype(fp8)
    in_maps = []
    for c in range(N_CORES):
        zc = np.roll(zn, -RPC * c, axis=0)[:NCOLS]
        # [d, col] -> [p, group, k, j] with d = 128*k + p, col = 1024*g + j
        znt = zc.T.astype(fp8).reshape(2, 128, NGRP, RPC)
        znt8 = np.ascontiguousarray(znt.transpose(1, 2, 0, 3))
        in_maps.append(
            {"znt8": znt8, "eye": eye, "eye8n": eye8n, "eyesh": eyesh, "ramp": ramp}
        )
    nc = _get_nc()
    res = run_bass_kernel_spmd(nc, in_maps, list(range(N_CORES)), trace=TRACE)
    LAST_EXEC_NS = res.exec_time_ns
    LAST_RESULTS = res

    # ---- gather / unshard: assemble global denominator & positives ----
    den = np.zeros(TWO_N, dtype=np.float64)
    pos = np.zeros(TWO_N, dtype=np.float64)
    idx = np.arange(NCOLS)
    for c in range(N_CORES):
        r = res.results[c]
        rows = RPC * c + np.arange(RPC)
        den[rows] += r["den_direct"].astype(np.float64).T.reshape(-1)
        pos[rows] += r["pos"].astype(np.float64).T.reshape(-1)
        cs = r["colsum"].astype(np.float64).sum(axis=0)
        cs[0:128] = 0.0          # never-written edges (diagonal subtiles
        cs[GA_END : GA_END + 128] = 0.0  # are covered by direct row sums)
        den[(RPC * c + idx) % TWO_N] += cs
    loss = np.mean(np.log(den) - pos)
    return np.array(loss, dtype=np.float32)
